# revision 6
# baseline (speedup 1.0000x reference)
"""DecoderTreeRNN Trainium2 kernel (v2).

h0 = relu(encoding); expand a depth-`depth` binary tree with two zero-input
GRU cells (left/right); project every leaf hidden state with W_out and take
log_softmax over the vocab.

Pure data parallel over 8 NeuronCores (batch sharded), weights replicated.

v2 structure (per core, Bc=32, H=512, V=10000, depth=6):
  - batch split into two sub-batches of 16; the second sub-batch's GRU tree
    is emitted in two blocks between the first sub-batch's projection chunks
    so the output DMA (the largest fixed cost, ~82 MB/core) overlaps tree
    expansion instead of waiting for it.
  - projection matmul runs fp8e4 DoubleRow (both operands fp8, K-chunk pairs
    in AP dim 1) -> ~2x fewer PE instructions than bf16.
  - logits are drained PSUM->SBUF as bf16 (y) with the b_out add fused on
    VectorE; exp runs once per 128-token chunk on ScalarE with accum_out
    giving the softmax sum directly (main exp output is discarded into the
    fp32 finalize buffer, which the subtract overwrites right after).
  - final (logit - logsumexp) is split across three engines per chunk:
    GPSIMD tensor_scalar, VectorE tensor_scalar, ScalarE Identity+bias.
  - output stores are one DMA per (chunk, span) over all 128 partitions via
    a DRAM-side rearrange (HWDGE dma_start costs ~1.1us of Sync time each,
    so fewer+bigger is better).
"""

import os
import sys
from contextlib import ExitStack

import numpy as np

for _p in ("/opt/trn_rl_repo", "/root/.axon_site/_ro/trn_rl_repo"):
    if os.path.isdir(_p) and _p not in sys.path:
        sys.path.insert(0, _p)

import ml_dtypes

N_CORES = 8
P = 128
NBF = 512    # fp32 elements per PSUM bank
PVG = 1024   # projection PSUM group width (2 banks, 2 bufs)
TTG = 256    # GRU token tile (PSUM [P,KH,TTG] fp32 = 2 banks, 2 bufs)

# final-subtract span split (fractions of V): gpsimd / vector / scalar
FR_P = 0.45
FR_V = 0.27

# Set by test harness to capture a profile on the next kernel() call.
TRACE = False
SIM_SAFE_DMA = False  # retained for test_sim compat (v2 DMAs are sim-safe)
LAST_EXEC_NS = None
LAST_RESULTS = None

_COMPILE_CACHE = {}


def _bitrev(x, bits):
    r = 0
    for _ in range(bits):
        r = (r << 1) | (x & 1)
        x >>= 1
    return r


def _numpy_reference(encoding, W_hh_l, b_ih_l, b_hh_l, W_hh_r, b_ih_r, b_hh_r,
                     W_out, b_out, depth):
    def gru(h, W, b_ih, b_hh):
        Hd = h.shape[-1]
        gh = h @ W.T + b_hh
        r = 1.0 / (1.0 + np.exp(-(b_ih[:Hd] + gh[..., :Hd])))
        z = 1.0 / (1.0 + np.exp(-(b_ih[Hd:2 * Hd] + gh[..., Hd:2 * Hd])))
        n = np.tanh(b_ih[2 * Hd:] + r * gh[..., 2 * Hd:])
        return (1.0 - z) * n + z * h

    h = np.maximum(encoding, 0.0)[:, None, :]
    for _ in range(depth):
        left = gru(h, W_hh_l, b_ih_l, b_hh_l)
        right = gru(h, W_hh_r, b_ih_r, b_hh_r)
        h = np.stack([left, right], axis=2).reshape(h.shape[0], -1, h.shape[-1])
    logits = h @ W_out.T + b_out
    m = logits.max(axis=-1, keepdims=True)
    e = np.exp(logits - m)
    return (logits - m) - np.log(e.sum(axis=-1, keepdims=True))


def _patch_act_tables(bacc, mybir):
    """Constrain the ACT table-set chooser: GRU funcs in one set, projection
    funcs in another, so each phase block costs at most one table load."""
    from concourse import hw_specs
    AF = mybir.ActivationFunctionType
    orig = hw_specs.get_activation_tables
    if getattr(bacc.get_activation_tables, "_treernn_patch", False):
        return
    keep = {
        "sigmoid_and_others": {AF.Sigmoid, AF.Tanh, AF.Relu},
        "natural_log_exp_and_others": {AF.Exp, AF.Ln, AF.Identity, AF.Copy},
    }
    controlled = set().union(*keep.values())

    def patched(arch):
        tabs = {k: set(v) for k, v in orig(arch).items()}
        for name, s in tabs.items():
            s -= controlled
            s |= keep.get(name, set())
        return tabs

    patched._treernn_patch = True
    bacc.get_activation_tables = patched


def _build(Bc, H, V, depth):
    """Build + compile the single-core SPMD program (identical on all cores)."""
    import concourse.bass as bass  # noqa: F401
    import concourse.tile as tile
    from concourse import bacc, mybir
    import bass_rust as _br

    f32 = mybir.dt.float32
    bf16 = mybir.dt.bfloat16
    fp8 = mybir.dt.float8e4
    AF = mybir.ActivationFunctionType
    OP = mybir.AluOpType
    DR = mybir.MatmulPerfMode.DoubleRow
    _patch_act_tables(bacc, mybir)

    KH = H // P
    H3 = 3 * H
    L = 1 << depth
    TOK = Bc * L
    NTC = TOK // P

    # sub-batches: (batch offset, size, chunk base)
    if Bc == 32 and depth >= 4:
        SUBS = [(0, 16, 0), (16, 16, 16 * L // P)]
    else:
        SUBS = [(0, Bc, 0)]
    NLL = {S: P // S for _, S, _ in SUBS}  # leaves per 128-token chunk

    def _chunks(width):
        out, pos = [], 0
        while pos < V:
            w = min(width, V - pos)
            out.append((pos, w))
            pos += w
        return out

    pgroups = _chunks(PVG)

    # subtract spans: [0,a)=gpsimd  [a,b)=vector  [b,V)=scalar
    sp_a = max(16, int(V * FR_P)) & ~15
    sp_b = sp_a + (max(16, int(V * FR_V)) & ~15)
    assert sp_b < V

    nc = bacc.Bacc("TRN2", target_bir_lowering=False, debug=False,
                   num_devices=N_CORES)

    enc_d = nc.dram_tensor("enc_t", [P, KH, Bc], f32, kind="ExternalInput").ap()
    whh_d = {s: nc.dram_tensor(f"whht_{s}", [P, KH, H3], bf16,
                               kind="ExternalInput").ap() for s in "lr"}
    bias_d = {s: nc.dram_tensor(f"bias_{s}", [P, 4 * KH], f32,
                                kind="ExternalInput").ap() for s in "lr"}
    wout_d = nc.dram_tensor("woutt", [P, KH, V], fp8, kind="ExternalInput").ap()
    bout_d = nc.dram_tensor("bout", [P, V], bf16, kind="ExternalInput").ap()
    out_d = nc.dram_tensor("out", [Bc, L, V], f32, kind="ExternalOutput").ap()

    with tile.TileContext(nc) as tc, ExitStack() as ctx:
        constp = ctx.enter_context(tc.tile_pool(name="const", bufs=1))
        ht2p = ctx.enter_context(tc.tile_pool(name="ht2", bufs=1))
        wvp = ctx.enter_context(tc.tile_pool(name="wv", bufs=1))
        ghp = ctx.enter_context(tc.tile_pool(name="gh", bufs=1))
        gap = ctx.enter_context(tc.tile_pool(name="gact", bufs=1))
        yp = ctx.enter_context(tc.tile_pool(name="ypool", bufs=1))
        yfp = ctx.enter_context(tc.tile_pool(name="yfpool", bufs=1))
        stp = ctx.enter_context(tc.tile_pool(name="stat", bufs=1))
        gpp = ctx.enter_context(tc.tile_pool(name="gpsum", bufs=1, space="PSUM"))
        ppp = ctx.enter_context(tc.tile_pool(name="ppsum", bufs=1, space="PSUM"))

        ht2 = ht2p.tile([P, KH, TOK], fp8)

        # ---- constants ----
        bsig, btanh, bnhh = {}, {}, {}
        for s in "lr":
            bt = constp.tile([P, 4 * KH], f32, name=f"bias{s}")
            nc.sync.dma_start(out=bt, in_=bias_d[s])
            bsig[s] = bt[:, :2 * KH]
            btanh[s] = bt[:, 2 * KH:3 * KH]
            bnhh[s] = bt[:, 3 * KH:]
        whh = {}
        for s in "lr":
            w = constp.tile([P, KH, H3], bf16, name=f"whh{s}")
            nc.sync.dma_start(out=w, in_=whh_d[s])
            whh[s] = w
        enc_sb = constp.tile([P, KH, Bc], f32, name="enc_stage")
        nc.sync.dma_start(out=enc_sb, in_=enc_d)
        h0 = constp.tile([P, KH, Bc], bf16, name="h0")
        nc.scalar.activation(out=h0, in_=enc_sb, func=AF.Relu)

        bout_sb = constp.tile([P, V], bf16, name="bout_sb")
        nc.sync.dma_start(out=bout_sb, in_=bout_d)
        wv = []
        for vg, (vs, vw) in enumerate(pgroups):
            wt = wvp.tile([P, KH, vw], fp8, name=f"wv{vg}", tag=f"wv{vg}")
            nc.sync.dma_start(out=wt, in_=wout_d[:, :, vs:vs + vw])
            wv.append(wt)

        # ---- GRU tree ----
        def emit_gru_level(s0, S, cbase, lvl, h_cur):
            t = S << lvl
            last = (lvl == depth - 1)
            nll = NLL[S]
            h_nxt = None
            if not last:
                h_nxt = ghp.tile([P, KH, 2 * t], bf16, tag="h", bufs=2,
                                 name=f"h_{s0}_{lvl + 1}")
            for si, s in enumerate("lr"):
                soff = si * t
                for t0 in range(0, t, TTG):
                    tt = min(TTG, t - t0)
                    hs = h_cur[:, :, t0:t0 + tt]
                    r_sb = gap.tile([P, KH, TTG], bf16, tag="r", bufs=2,
                                    name="g_r")[:, :, :tt]
                    z_sb = gap.tile([P, KH, TTG], bf16, tag="z", bufs=2,
                                    name="g_z")[:, :, :tt]
                    n_sb = gap.tile([P, KH, TTG], bf16, tag="n", bufs=2,
                                    name="g_n")[:, :, :tt]
                    d_sb = gap.tile([P, KH, TTG], bf16, tag="d", bufs=2,
                                    name="g_d")[:, :, :tt]
                    for gi in range(3):  # r, z, n
                        ps = gpp.tile([P, KH, TTG], f32, tag="g", bufs=2,
                                      name="g_ps")[:, :, :tt]
                        for gc in range(KH):
                            col = gi * H + gc * P
                            for k in range(KH):
                                nc.tensor.matmul(
                                    ps[:, gc, :],
                                    lhsT=whh[s][:, k, col:col + P],
                                    rhs=hs[:, k, :],
                                    start=(k == 0), stop=(k == KH - 1))
                        if gi == 0:
                            for gc in range(KH):
                                nc.scalar.activation(
                                    out=r_sb[:, gc, :], in_=ps[:, gc, :],
                                    func=AF.Sigmoid,
                                    bias=bsig[s][:, gc:gc + 1])
                        elif gi == 1:
                            for gc in range(KH):
                                nc.scalar.activation(
                                    out=z_sb[:, gc, :], in_=ps[:, gc, :],
                                    func=AF.Sigmoid,
                                    bias=bsig[s][:, KH + gc:KH + gc + 1])
                        else:
                            for gc in range(KH):
                                # n_pre = (gh_n + b_hh_n) * r
                                nc.vector.scalar_tensor_tensor(
                                    out=n_sb[:, gc, :], in0=ps[:, gc, :],
                                    scalar=bnhh[s][:, gc:gc + 1],
                                    in1=r_sb[:, gc, :],
                                    op0=OP.add, op1=OP.mult)
                            for gc in range(KH):
                                nc.scalar.activation(
                                    out=n_sb[:, gc, :], in_=n_sb[:, gc, :],
                                    func=AF.Tanh,
                                    bias=btanh[s][:, gc:gc + 1])
                    # h' = n + z * (h - n)
                    nc.vector.tensor_tensor(d_sb, hs, n_sb, OP.subtract)
                    nc.vector.tensor_tensor(d_sb, d_sb, z_sb, OP.mult)
                    if not last:
                        nc.vector.tensor_tensor(
                            h_nxt[:, :, soff + t0:soff + t0 + tt],
                            d_sb, n_sb, OP.add)
                    else:
                        # write leaves straight into ht2 (fp8), bit-reversed
                        # node order, tokens interleaved p = b*nll + ll
                        for i in range(tt // S):
                            node = (soff + t0) // S + i
                            leaf = _bitrev(node, depth)
                            tcp, ll = leaf // nll, leaf % nll
                            base = (cbase + tcp) * P + ll
                            nc.vector.tensor_tensor(
                                ht2[:, :, base:base + (S - 1) * nll + 1:nll],
                                d_sb[:, :, i * S:(i + 1) * S],
                                n_sb[:, :, i * S:(i + 1) * S], OP.add)
            return h_nxt

        def emit_gru_tree(sbi, levels):
            s0, S, cbase = SUBS[sbi]
            h = gru_state.get(sbi, h0[:, :, s0:s0 + S])
            for lvl in levels:
                h = emit_gru_level(s0, S, cbase, lvl, h)
            gru_state[sbi] = h

        gru_state = {}

        # ---- projection chunk ----
        # out view: [chunk, (b ll), V] per sub-batch
        oviews = {}
        for s0, S, cbase in SUBS:
            oviews[s0] = out_d[s0:s0 + S].rearrange(
                "b (lc ll) v -> lc b ll v", ll=NLL[S])

        pending = []  # delayed vector-span subtracts from the previous chunk

        def out_dma(sb, tcp, yf, v0, v1):
            s0 = SUBS[sb][0]
            nc.sync.dma_start(out=oviews[s0][tcp][:, :, v0:v1],
                              in_=yf[:, v0:v1])

        def emit_proj_chunk(sb, tcp, fine=False):
            s0, S, cbase = SUBS[sb]
            gci = cbase + tcp
            y = yp.tile([P, V], bf16, tag="y", name="y", bufs=2)
            cs = stp.tile([P, 4], f32, tag="cs", name="cs", bufs=3)
            for vg, (vs, vw) in enumerate(pgroups):
                ps = ppp.tile([P, PVG], f32, tag="pp", name="p_vg",
                              bufs=2)[:, :vw]
                for vt0 in range(0, vw, NBF):
                    w = min(NBF, vw - vt0)
                    for j in range(2):
                        nc.tensor.matmul(
                            ps[:, vt0:vt0 + w],
                            lhsT=ht2[:, 2 * j:2 * j + 2, gci * P:(gci + 1) * P],
                            rhs=wv[vg][:, 2 * j:2 * j + 2, vt0:vt0 + w],
                            start=(j == 0), stop=(j == 1), perf_mode=DR)
                tt = nc.vector.tensor_tensor(
                    y[:, vs:vs + vw], ps, bout_sb[:, vs:vs + vw], OP.add)
                if vg == 0:
                    for fn in pending:
                        fn(tt)
                    pending.clear()
            yf = yfp.tile([P, V], f32, tag="yf", name="yf")
            if fine:
                h1 = (V // 2) & ~15
                nc.scalar.activation(out=yf[:, :h1], in_=y[:, :h1],
                                     func=AF.Exp, accum_out=cs[:, 0:1])
                nc.scalar.activation(out=yf[:, h1:], in_=y[:, h1:],
                                     func=AF.Exp, accum_out=cs[:, 3:4])
                nc.vector.tensor_tensor(cs[:, 0:1], cs[:, 0:1], cs[:, 3:4],
                                        OP.add)
            else:
                nc.scalar.activation(out=yf, in_=y, func=AF.Exp,
                                     accum_out=cs[:, 0:1])
            nc.scalar.activation(out=cs[:, 1:2], in_=cs[:, 0:1], func=AF.Ln)
            nc.vector.tensor_scalar(out=cs[:, 2:3], in0=cs[:, 1:2],
                                    scalar1=-1.0, scalar2=None, op0=OP.mult)

            def sub_p(a, b):
                nc.gpsimd.tensor_scalar(out=yf[:, a:b], in0=y[:, a:b],
                                        scalar1=cs[:, 1:2], scalar2=None,
                                        op0=OP.subtract)
                out_dma(sb, tcp, yf, a, b)

            def sub_s(a, b):
                nc.scalar.activation(out=yf[:, a:b], in_=y[:, a:b],
                                     func=AF.Identity, bias=cs[:, 2:3])
                out_dma(sb, tcp, yf, a, b)

            def sub_v(a, b, after=None):
                ins = nc.vector.tensor_scalar(out=yf[:, a:b], in0=y[:, a:b],
                                              scalar1=cs[:, 1:2], scalar2=None,
                                              op0=OP.subtract)
                if after is not None:
                    _br.add_dep_helper(
                        ins.ins, after.ins, sync=False,
                        reason="tail subtract yields to next chunk drain")
                out_dma(sb, tcp, yf, a, b)

            if fine:
                # last chunk: small alternating pieces so the kernel-end
                # drain isn't one big op + one big DMA
                nq = 6
                qs = [(V * i // nq) & ~15 for i in range(nq)] + [V]
                eng = [sub_p, sub_v, sub_s, sub_p, sub_v, sub_s]
                for i in range(nq):
                    eng[i](qs[i], qs[i + 1])
            else:
                sub_p(0, sp_a)
                sub_s(sp_b, V)
                pending.append(lambda tt, a=sp_a, b=sp_b: sub_v(a, b, after=tt))

        # ---- emission schedule ----
        emit_gru_tree(0, range(depth))
        n0 = SUBS[0][1] * L // P  # chunks in sub-batch 0
        if len(SUBS) > 1:
            # interleave sub-batch 1's tree between sub-batch 0's chunks in
            # two blocks (one ACT table-set switch each way per block)
            blocks = {min(2, n0 - 1): range(depth - 1),
                      min(5, n0 - 1): [depth - 1]}
            for tcp in range(n0):
                emit_proj_chunk(0, tcp)
                if tcp in blocks:
                    emit_gru_tree(1, blocks[tcp])
            n1 = SUBS[1][1] * L // P
            for tcp in range(n1):
                emit_proj_chunk(1, tcp, fine=(tcp == n1 - 1))
        else:
            for tcp in range(n0):
                emit_proj_chunk(0, tcp, fine=(tcp == n0 - 1))
        assert not pending

    nc.compile()
    return nc


def _packed_bias(b_ih, b_hh, H, KH):
    """[P, 4*KH]: sigmoid biases (b_ih+b_hh for r,z), tanh bias (b_ih_n),
    and the pre-multiply n-gate bias (b_hh_n), per 128-row chunk."""
    P = 128
    sig = (b_ih + b_hh)[:2 * H].reshape(2 * KH, P).T
    tanh = b_ih[2 * H:].reshape(KH, P).T
    nhh = b_hh[2 * H:].reshape(KH, P).T
    return np.ascontiguousarray(np.concatenate([sig, tanh, nhh], axis=1))


def _get_compiled(Bc, H, V, depth):
    key = (Bc, H, V, depth)
    if key not in _COMPILE_CACHE:
        _COMPILE_CACHE[key] = _build(Bc, H, V, depth)
    return _COMPILE_CACHE[key]


def kernel(encoding, W_hh_l, b_ih_l, b_hh_l, W_hh_r, b_ih_r, b_hh_r,
           W_out, b_out, depth):
    global LAST_EXEC_NS, LAST_RESULTS
    encoding = np.asarray(encoding, np.float32)
    W_hh = {"l": np.asarray(W_hh_l, np.float32), "r": np.asarray(W_hh_r, np.float32)}
    b_ih = {"l": np.asarray(b_ih_l, np.float32), "r": np.asarray(b_ih_r, np.float32)}
    b_hh = {"l": np.asarray(b_hh_l, np.float32), "r": np.asarray(b_hh_r, np.float32)}
    W_out = np.asarray(W_out, np.float32)
    b_out = np.asarray(b_out, np.float32)
    depth = int(depth)

    B, H = encoding.shape
    V = W_out.shape[0]
    tok = (B // N_CORES) * (1 << depth) if B % N_CORES == 0 else 0
    if (depth < 1 or B % N_CORES or H % P or P % (B // N_CORES)
            or (tok % P != 0 and tok > P)):
        return _numpy_reference(encoding, W_hh["l"], b_ih["l"], b_hh["l"],
                                W_hh["r"], b_ih["r"], b_hh["r"],
                                W_out, b_out, depth).astype(np.float32)

    Bc = B // N_CORES
    KH = H // P
    bf16 = ml_dtypes.bfloat16
    fp8 = ml_dtypes.float8_e4m3

    nc = _get_compiled(Bc, H, V, depth)

    # device layouts are [P(partition), KH, x]: H index = k*P + p -> axes (p, k)
    woutt = np.ascontiguousarray(
        np.clip(W_out.T, -240, 240).astype(fp8).reshape(KH, P, V).transpose(1, 0, 2))
    bout_b = np.ascontiguousarray(
        np.broadcast_to(b_out.astype(bf16)[None, :], (P, V)))
    shared = {"woutt": woutt, "bout": bout_b}
    for s in "lr":
        shared[f"whht_{s}"] = np.ascontiguousarray(
            W_hh[s].T.astype(bf16).reshape(KH, P, 3 * H).transpose(1, 0, 2))
        shared[f"bias_{s}"] = _packed_bias(b_ih[s], b_hh[s], H, KH)

    encT = encoding.T  # [H, B]
    in_maps = []
    for c in range(N_CORES):
        enc_c = np.ascontiguousarray(
            encT[:, c * Bc:(c + 1) * Bc].reshape(KH, P, Bc).transpose(1, 0, 2))
        in_maps.append({"enc_t": enc_c, **shared})

    from concourse import bass_utils
    kw = {}
    if TRACE:
        kw["tmpdir"] = os.environ.get("BASS_TRACE_DIR") or None
    res = bass_utils.run_bass_kernel_spmd(
        nc, in_maps, core_ids=list(range(N_CORES)), trace=TRACE, **kw)
    LAST_EXEC_NS = res.exec_time_ns
    LAST_RESULTS = res
    out = np.concatenate([r["out"] for r in res.results], axis=0)
    return np.ascontiguousarray(out.astype(np.float32))


# revision 11
# speedup vs baseline: 2.2280x; 2.2280x over previous
"""DecoderTreeRNN Trainium2 kernel (v2).

h0 = relu(encoding); expand a depth-`depth` binary tree with two zero-input
GRU cells (left/right); project every leaf hidden state with W_out and take
log_softmax over the vocab.

Pure data parallel over 8 NeuronCores (batch sharded), weights replicated.

v2 structure (per core, Bc=32, H=512, V=10000, depth=6):
  - batch split into two sub-batches of 16; the second sub-batch's GRU tree
    is emitted in two blocks between the first sub-batch's projection chunks
    so the output DMA (the largest fixed cost, ~82 MB/core) overlaps tree
    expansion instead of waiting for it.
  - projection matmul runs fp8e4 DoubleRow (both operands fp8, K-chunk pairs
    in AP dim 1) -> ~2x fewer PE instructions than bf16.
  - logits are drained PSUM->SBUF as bf16 (y) with the b_out add fused on
    VectorE; exp runs once per 128-token chunk on ScalarE with accum_out
    giving the softmax sum directly (main exp output is discarded into the
    fp32 finalize buffer, which the subtract overwrites right after).
  - final (logit - logsumexp) is split across three engines per chunk:
    GPSIMD tensor_scalar, VectorE tensor_scalar, ScalarE Identity+bias.
  - output stores are one DMA per (chunk, span) over all 128 partitions via
    a DRAM-side rearrange (HWDGE dma_start costs ~1.1us of Sync time each,
    so fewer+bigger is better).
"""

import os
import sys
from contextlib import ExitStack

import numpy as np

for _p in ("/opt/trn_rl_repo", "/root/.axon_site/_ro/trn_rl_repo"):
    if os.path.isdir(_p) and _p not in sys.path:
        sys.path.insert(0, _p)

import ml_dtypes

N_CORES = 8
P = 128
NBF = 512    # fp32 elements per PSUM bank
PVG = 1024   # projection PSUM group width (2 banks, 2 bufs)
TTG = 256    # GRU token tile (PSUM [P,KH,TTG] fp32 = 2 banks, 2 bufs)

# vocab split: cols [0, XV) drain to fp32 y32 (VectorE fp32 tensor_scalar
# subtract — the only fast DVE path with fp32 out); cols [XV, V) drain to
# bf16 yb (ScalarE Identity+bias subtract, line-rate with dtype convert).
# Must be a multiple of PVG so drains don't straddle the boundary.
XV_GROUPS = 5

# Set by test harness to capture a profile on the next kernel() call.
TRACE = False
SIM_SAFE_DMA = False  # retained for test_sim compat (v2 DMAs are sim-safe)
LAST_EXEC_NS = None
LAST_RESULTS = None

_COMPILE_CACHE = {}


def _bitrev(x, bits):
    r = 0
    for _ in range(bits):
        r = (r << 1) | (x & 1)
        x >>= 1
    return r


def _numpy_reference(encoding, W_hh_l, b_ih_l, b_hh_l, W_hh_r, b_ih_r, b_hh_r,
                     W_out, b_out, depth):
    def gru(h, W, b_ih, b_hh):
        Hd = h.shape[-1]
        gh = h @ W.T + b_hh
        r = 1.0 / (1.0 + np.exp(-(b_ih[:Hd] + gh[..., :Hd])))
        z = 1.0 / (1.0 + np.exp(-(b_ih[Hd:2 * Hd] + gh[..., Hd:2 * Hd])))
        n = np.tanh(b_ih[2 * Hd:] + r * gh[..., 2 * Hd:])
        return (1.0 - z) * n + z * h

    h = np.maximum(encoding, 0.0)[:, None, :]
    for _ in range(depth):
        left = gru(h, W_hh_l, b_ih_l, b_hh_l)
        right = gru(h, W_hh_r, b_ih_r, b_hh_r)
        h = np.stack([left, right], axis=2).reshape(h.shape[0], -1, h.shape[-1])
    logits = h @ W_out.T + b_out
    m = logits.max(axis=-1, keepdims=True)
    e = np.exp(logits - m)
    return (logits - m) - np.log(e.sum(axis=-1, keepdims=True))


def _patch_act_tables(bacc, mybir):
    """Constrain the ACT table-set chooser: GRU funcs in one set, projection
    funcs in another, so each phase block costs at most one table load."""
    from concourse import hw_specs
    AF = mybir.ActivationFunctionType
    orig = hw_specs.get_activation_tables
    if getattr(bacc.get_activation_tables, "_treernn_patch", False):
        return
    keep = {
        "sigmoid_and_others": {AF.Sigmoid, AF.Tanh, AF.Relu},
        "natural_log_exp_and_others": {AF.Exp, AF.Ln, AF.Identity, AF.Copy},
    }
    controlled = set().union(*keep.values())

    def patched(arch):
        tabs = {k: set(v) for k, v in orig(arch).items()}
        for name, s in tabs.items():
            s -= controlled
            s |= keep.get(name, set())
        return tabs

    patched._treernn_patch = True
    bacc.get_activation_tables = patched


def _build(Bc, H, V, depth):
    """Build + compile the single-core SPMD program (identical on all cores)."""
    import concourse.bass as bass  # noqa: F401
    import concourse.tile as tile
    from concourse import bacc, mybir
    import bass_rust as _br

    f32 = mybir.dt.float32
    bf16 = mybir.dt.bfloat16
    fp8 = mybir.dt.float8e4
    AF = mybir.ActivationFunctionType
    OP = mybir.AluOpType
    DR = mybir.MatmulPerfMode.DoubleRow
    _patch_act_tables(bacc, mybir)

    KH = H // P
    H3 = 3 * H
    L = 1 << depth
    TOK = Bc * L
    NTC = TOK // P

    # sub-batches: (batch offset, size, chunk base)
    if Bc == 32 and depth >= 4:
        SUBS = [(0, 16, 0), (16, 16, 16 * L // P)]
    else:
        SUBS = [(0, Bc, 0)]
    NLL = {S: P // S for _, S, _ in SUBS}  # leaves per 128-token chunk

    def _chunks(width):
        out, pos = [], 0
        while pos < V:
            w = min(width, V - pos)
            out.append((pos, w))
            pos += w
        return out

    pgroups = _chunks(PVG)
    XV = min(XV_GROUPS * PVG, pgroups[-1][0])  # fp32-region width

    nc = bacc.Bacc("TRN2", target_bir_lowering=False, debug=False,
                   num_devices=N_CORES)

    enc_d = nc.dram_tensor("enc_t", [P, KH, Bc], f32, kind="ExternalInput").ap()
    whh_d = {s: nc.dram_tensor(f"whht_{s}", [P, KH, H3], bf16,
                               kind="ExternalInput").ap() for s in "lr"}
    bias_d = {s: nc.dram_tensor(f"bias_{s}", [P, 4 * KH], f32,
                                kind="ExternalInput").ap() for s in "lr"}
    wout_d = nc.dram_tensor("woutt", [P, KH, V], fp8, kind="ExternalInput").ap()
    bout_d = nc.dram_tensor("bout", [P, V], bf16, kind="ExternalInput").ap()
    out_d = nc.dram_tensor("out", [Bc, L, V], f32, kind="ExternalOutput").ap()

    with tile.TileContext(nc) as tc, ExitStack() as ctx:
        constp = ctx.enter_context(tc.tile_pool(name="const", bufs=1))
        ht2p = ctx.enter_context(tc.tile_pool(name="ht2", bufs=1))
        wvp = ctx.enter_context(tc.tile_pool(name="wv", bufs=1))
        ghp = ctx.enter_context(tc.tile_pool(name="gh", bufs=1))
        gap = ctx.enter_context(tc.tile_pool(name="gact", bufs=1))
        yp = ctx.enter_context(tc.tile_pool(name="ypool", bufs=1))
        yfp = ctx.enter_context(tc.tile_pool(name="yfpool", bufs=1))
        stp = ctx.enter_context(tc.tile_pool(name="stat", bufs=1))
        gpp = ctx.enter_context(tc.tile_pool(name="gpsum", bufs=1, space="PSUM"))
        ppp = ctx.enter_context(tc.tile_pool(name="ppsum", bufs=1, space="PSUM"))

        ht2 = ht2p.tile([P, KH, TOK], fp8)

        # ---- constants ----
        bsig, btanh, bnhh = {}, {}, {}
        for s in "lr":
            bt = constp.tile([P, 4 * KH], f32, name=f"bias{s}")
            nc.sync.dma_start(out=bt, in_=bias_d[s])
            bsig[s] = bt[:, :2 * KH]
            btanh[s] = bt[:, 2 * KH:3 * KH]
            bnhh[s] = bt[:, 3 * KH:]
        whh = {}
        for s in "lr":
            w = constp.tile([P, KH, H3], bf16, name=f"whh{s}")
            nc.sync.dma_start(out=w, in_=whh_d[s])
            whh[s] = w
        enc_sb = constp.tile([P, KH, Bc], f32, name="enc_stage")
        nc.sync.dma_start(out=enc_sb, in_=enc_d)
        h0 = constp.tile([P, KH, Bc], bf16, name="h0")
        nc.scalar.activation(out=h0, in_=enc_sb, func=AF.Relu)

        bout_sb = constp.tile([P, V], bf16, name="bout_sb")
        nc.sync.dma_start(out=bout_sb, in_=bout_d)
        wv = []
        for vg, (vs, vw) in enumerate(pgroups):
            wt = wvp.tile([P, KH, vw], fp8, name=f"wv{vg}", tag=f"wv{vg}")
            nc.sync.dma_start(out=wt, in_=wout_d[:, :, vs:vs + vw])
            wv.append(wt)

        # ---- GRU tree ----
        def emit_gru_level(s0, S, cbase, lvl, h_cur):
            t = S << lvl
            last = (lvl == depth - 1)
            nll = NLL[S]
            h_nxt = None
            if not last:
                h_nxt = ghp.tile([P, KH, 2 * t], bf16, tag="h", bufs=2,
                                 name=f"h_{s0}_{lvl + 1}")
            for si, s in enumerate("lr"):
                soff = si * t
                for t0 in range(0, t, TTG):
                    tt = min(TTG, t - t0)
                    hs = h_cur[:, :, t0:t0 + tt]
                    r_sb = gap.tile([P, KH, TTG], bf16, tag="r", bufs=2,
                                    name="g_r")[:, :, :tt]
                    z_sb = gap.tile([P, KH, TTG], bf16, tag="z", bufs=2,
                                    name="g_z")[:, :, :tt]
                    n_sb = gap.tile([P, KH, TTG], bf16, tag="n", bufs=2,
                                    name="g_n")[:, :, :tt]
                    d_sb = gap.tile([P, KH, TTG], bf16, tag="d", bufs=2,
                                    name="g_d")[:, :, :tt]
                    for gi in range(3):  # r, z, n
                        ps = gpp.tile([P, KH, TTG], f32, tag="g", bufs=2,
                                      name="g_ps")[:, :, :tt]
                        for gc in range(KH):
                            col = gi * H + gc * P
                            for k in range(KH):
                                nc.tensor.matmul(
                                    ps[:, gc, :],
                                    lhsT=whh[s][:, k, col:col + P],
                                    rhs=hs[:, k, :],
                                    start=(k == 0), stop=(k == KH - 1))
                        if gi == 0:
                            for gc in range(KH):
                                nc.scalar.activation(
                                    out=r_sb[:, gc, :], in_=ps[:, gc, :],
                                    func=AF.Sigmoid,
                                    bias=bsig[s][:, gc:gc + 1])
                        elif gi == 1:
                            for gc in range(KH):
                                nc.scalar.activation(
                                    out=z_sb[:, gc, :], in_=ps[:, gc, :],
                                    func=AF.Sigmoid,
                                    bias=bsig[s][:, KH + gc:KH + gc + 1])
                        else:
                            for gc in range(KH):
                                # n_pre = (gh_n + b_hh_n) * r
                                nc.vector.scalar_tensor_tensor(
                                    out=n_sb[:, gc, :], in0=ps[:, gc, :],
                                    scalar=bnhh[s][:, gc:gc + 1],
                                    in1=r_sb[:, gc, :],
                                    op0=OP.add, op1=OP.mult)
                            for gc in range(KH):
                                nc.scalar.activation(
                                    out=n_sb[:, gc, :], in_=n_sb[:, gc, :],
                                    func=AF.Tanh,
                                    bias=btanh[s][:, gc:gc + 1])
                    # h' = n + z * (h - n); the two intermediate TTs run on
                    # GPSIMD (idle otherwise; bf16->bf16 TT is its one fast
                    # elementwise path) to take load off VectorE
                    nc.gpsimd.tensor_tensor(d_sb, hs, n_sb, OP.subtract)
                    nc.gpsimd.tensor_tensor(d_sb, d_sb, z_sb, OP.mult)
                    if not last:
                        nc.vector.tensor_tensor(
                            h_nxt[:, :, soff + t0:soff + t0 + tt],
                            d_sb, n_sb, OP.add)
                    else:
                        # write leaves straight into ht2 (fp8), bit-reversed
                        # node order, tokens interleaved p = b*nll + ll
                        for i in range(tt // S):
                            node = (soff + t0) // S + i
                            leaf = _bitrev(node, depth)
                            tcp, ll = leaf // nll, leaf % nll
                            base = (cbase + tcp) * P + ll
                            nc.vector.tensor_tensor(
                                ht2[:, :, base:base + (S - 1) * nll + 1:nll],
                                d_sb[:, :, i * S:(i + 1) * S],
                                n_sb[:, :, i * S:(i + 1) * S], OP.add)
            return h_nxt

        def emit_gru_tree(sbi, levels):
            s0, S, cbase = SUBS[sbi]
            h = gru_state.get(sbi, h0[:, :, s0:s0 + S])
            for lvl in levels:
                h = emit_gru_level(s0, S, cbase, lvl, h)
            gru_state[sbi] = h

        gru_state = {}

        # ---- projection chunk ----
        # out view: [chunk, (b ll), V] per sub-batch
        oviews = {}
        for s0, S, cbase in SUBS:
            oviews[s0] = out_d[s0:s0 + S].rearrange(
                "b (lc ll) v -> lc b ll v", ll=NLL[S])

        pending = []  # delayed vector-span subtracts from the previous chunk

        def out_dma(sb, tcp, src, v0, v1, off=0):
            s0 = SUBS[sb][0]
            nc.sync.dma_start(out=oviews[s0][tcp][:, :, v0:v1],
                              in_=src[:, v0 - off:v1 - off])

        def emit_proj_chunk(sb, tcp, fine=False):
            s0, S, cbase = SUBS[sb]
            gci = cbase + tcp
            y32 = yp.tile([P, XV], f32, tag="y32", name="y32", bufs=2)
            yb = yp.tile([P, V - XV], bf16, tag="yb", name="yb", bufs=2)
            yS = yfp.tile([P, V - XV], f32, tag="yS", name="yS")
            esc = yfp.tile([P, XV], bf16, tag="esc", name="esc")
            cs = stp.tile([P, 4], f32, tag="cs", name="cs", bufs=3)
            for vg, (vs, vw) in enumerate(pgroups):
                ps = ppp.tile([P, PVG], f32, tag="pp", name="p_vg",
                              bufs=2)[:, :vw]
                for vt0 in range(0, vw, NBF):
                    w = min(NBF, vw - vt0)
                    for j in range(2):
                        nc.tensor.matmul(
                            ps[:, vt0:vt0 + w],
                            lhsT=ht2[:, 2 * j:2 * j + 2, gci * P:(gci + 1) * P],
                            rhs=wv[vg][:, 2 * j:2 * j + 2, vt0:vt0 + w],
                            start=(j == 0), stop=(j == 1), perf_mode=DR)
                ydst = (y32[:, vs:vs + vw] if vs < XV
                        else yb[:, vs - XV:vs - XV + vw])
                tt = nc.vector.tensor_tensor(
                    ydst, ps, bout_sb[:, vs:vs + vw], OP.add)
                if vg == 0:
                    for fn in pending:
                        fn(tt)
                    pending.clear()
            nc.scalar.activation(out=esc, in_=y32, func=AF.Exp,
                                 accum_out=cs[:, 0:1])
            nc.scalar.activation(out=yS, in_=yb, func=AF.Exp,
                                 accum_out=cs[:, 3:4])
            nc.vector.tensor_tensor(cs[:, 0:1], cs[:, 0:1], cs[:, 3:4],
                                    OP.add)
            nc.scalar.activation(out=cs[:, 1:2], in_=cs[:, 0:1], func=AF.Ln)
            nc.vector.tensor_scalar(out=cs[:, 2:3], in0=cs[:, 1:2],
                                    scalar1=-1.0, scalar2=None, op0=OP.mult)

            def sub_s(a, b):
                # [XV, V): out = yb + (-c) with line-rate dtype convert
                nc.scalar.activation(out=yS[:, a - XV:b - XV],
                                     in_=yb[:, a - XV:b - XV],
                                     func=AF.Identity, bias=cs[:, 2:3])
                out_dma(sb, tcp, yS, a, b, off=XV)

            def sub_v(a, b, after=None):
                # [0, XV): in-place fp32 tensor_scalar (2x mode)
                ins = nc.vector.tensor_scalar(out=y32[:, a:b], in0=y32[:, a:b],
                                              scalar1=cs[:, 1:2], scalar2=None,
                                              op0=OP.subtract)
                if after is not None:
                    _br.add_dep_helper(
                        ins.ins, after.ins, sync=False,
                        reason="tail subtract yields to next chunk drain")
                out_dma(sb, tcp, y32, a, b)

            if fine:
                # last chunk: small alternating pieces so the kernel-end
                # drain isn't one big op + one big DMA
                for a, b in ((0, XV // 2), (XV // 2, XV)):
                    sub_v(a, b)
                h2 = (XV + (V - XV) // 2) & ~15
                for a, b in ((XV, h2), (h2, V)):
                    sub_s(a, b)
            else:
                sub_s(XV, V)
                pending.append(lambda tt: sub_v(0, XV, after=tt))

        # ---- emission schedule ----
        emit_gru_tree(0, range(depth))
        n0 = SUBS[0][1] * L // P  # chunks in sub-batch 0
        if len(SUBS) > 1:
            # interleave sub-batch 1's tree between sub-batch 0's chunks in
            # two blocks (one ACT table-set switch each way per block)
            blocks = {min(2, n0 - 1): range(depth - 1),
                      min(5, n0 - 1): [depth - 1]}
            for tcp in range(n0):
                emit_proj_chunk(0, tcp)
                if tcp in blocks:
                    emit_gru_tree(1, blocks[tcp])
            n1 = SUBS[1][1] * L // P
            for tcp in range(n1):
                emit_proj_chunk(1, tcp, fine=(tcp == n1 - 1))
        else:
            for tcp in range(n0):
                emit_proj_chunk(0, tcp, fine=(tcp == n0 - 1))
        assert not pending

    nc.compile()
    return nc


def _packed_bias(b_ih, b_hh, H, KH):
    """[P, 4*KH]: sigmoid biases (b_ih+b_hh for r,z), tanh bias (b_ih_n),
    and the pre-multiply n-gate bias (b_hh_n), per 128-row chunk."""
    P = 128
    sig = (b_ih + b_hh)[:2 * H].reshape(2 * KH, P).T
    tanh = b_ih[2 * H:].reshape(KH, P).T
    nhh = b_hh[2 * H:].reshape(KH, P).T
    return np.ascontiguousarray(np.concatenate([sig, tanh, nhh], axis=1))


def _get_compiled(Bc, H, V, depth):
    key = (Bc, H, V, depth)
    if key not in _COMPILE_CACHE:
        _COMPILE_CACHE[key] = _build(Bc, H, V, depth)
    return _COMPILE_CACHE[key]


def kernel(encoding, W_hh_l, b_ih_l, b_hh_l, W_hh_r, b_ih_r, b_hh_r,
           W_out, b_out, depth):
    global LAST_EXEC_NS, LAST_RESULTS
    encoding = np.asarray(encoding, np.float32)
    W_hh = {"l": np.asarray(W_hh_l, np.float32), "r": np.asarray(W_hh_r, np.float32)}
    b_ih = {"l": np.asarray(b_ih_l, np.float32), "r": np.asarray(b_ih_r, np.float32)}
    b_hh = {"l": np.asarray(b_hh_l, np.float32), "r": np.asarray(b_hh_r, np.float32)}
    W_out = np.asarray(W_out, np.float32)
    b_out = np.asarray(b_out, np.float32)
    depth = int(depth)

    B, H = encoding.shape
    V = W_out.shape[0]
    tok = (B // N_CORES) * (1 << depth) if B % N_CORES == 0 else 0
    if (depth < 1 or B % N_CORES or H % P or P % (B // N_CORES)
            or (tok % P != 0 and tok > P)):
        return _numpy_reference(encoding, W_hh["l"], b_ih["l"], b_hh["l"],
                                W_hh["r"], b_ih["r"], b_hh["r"],
                                W_out, b_out, depth).astype(np.float32)

    Bc = B // N_CORES
    KH = H // P
    bf16 = ml_dtypes.bfloat16
    fp8 = ml_dtypes.float8_e4m3

    nc = _get_compiled(Bc, H, V, depth)

    # device layouts are [P(partition), KH, x]: H index = k*P + p -> axes (p, k)
    woutt = np.ascontiguousarray(
        np.clip(W_out.T, -240, 240).astype(fp8).reshape(KH, P, V).transpose(1, 0, 2))
    bout_b = np.ascontiguousarray(
        np.broadcast_to(b_out.astype(bf16)[None, :], (P, V)))
    shared = {"woutt": woutt, "bout": bout_b}
    for s in "lr":
        shared[f"whht_{s}"] = np.ascontiguousarray(
            W_hh[s].T.astype(bf16).reshape(KH, P, 3 * H).transpose(1, 0, 2))
        shared[f"bias_{s}"] = _packed_bias(b_ih[s], b_hh[s], H, KH)

    encT = encoding.T  # [H, B]
    in_maps = []
    for c in range(N_CORES):
        enc_c = np.ascontiguousarray(
            encT[:, c * Bc:(c + 1) * Bc].reshape(KH, P, Bc).transpose(1, 0, 2))
        in_maps.append({"enc_t": enc_c, **shared})

    from concourse import bass_utils
    kw = {}
    if TRACE:
        kw["tmpdir"] = os.environ.get("BASS_TRACE_DIR") or None
    res = bass_utils.run_bass_kernel_spmd(
        nc, in_maps, core_ids=list(range(N_CORES)), trace=TRACE, **kw)
    LAST_EXEC_NS = res.exec_time_ns
    LAST_RESULTS = res
    out = np.concatenate([r["out"] for r in res.results], axis=0)
    return np.ascontiguousarray(out.astype(np.float32))
